# revision 8
# baseline (speedup 1.0000x reference)
"""Trainium2 Bass kernel for nn_EvroModel (dense MLP 256->64->16->4 + global softmax).

Contract: kernel(**inputs) takes FULL unsharded numpy inputs and returns the
FULL [262144, 4] float32 output. Internally shards the batch across 8
NeuronCores (data parallel), runs one SPMD Bass/Tile kernel with a single
scalar AllReduce for the global softmax denominator, and concatenates the
per-core output shards.

Math per core (rows = 32768 shard of x):
  h1 = relu(x @ wz1 + b1); h2 = tanh(h1 @ wz2 + b2); h3 = h2 @ wz3 + b3
  e  = exp(h3)            (global max subtraction skipped: |h3| <~ 10, exp
                           stays in f32 range; e/sum(e) is max-invariant)
  y  = e / allreduce_sum(e)

Layout strategy: compute in "transposed" activation layout (features on SBUF
partitions, batch on the free dim) so TensorE contracts over features and all
bias adds fuse into ScalarE activations as per-partition bias APs.  x tiles are
cast f32->bf16 during the DMA load (SWDGE cast) and transposed on TensorE
(bf16 transposes keep weight loads on the fast path).  exp's accum_out gives
per-partition softmax partials for free; a ones-matmul folds them to a scalar.
Output returns to natural layout via DVE 32x32 stream-transpose.
"""

import numpy as np

B = 262144
F = 256
H1 = 64
H2 = 16
C = 4
N_CORES = 8
BS = B // N_CORES  # 32768 rows per core

QROWS = 2048          # rows per DMA load ("quad" = 4 groups of 512)
GROUPS_PER_Q = 4      # 512-row groups per quad
GROUP = 512
CHUNKS_PER_G = 4      # 128-row chunks per group

_CACHE = {}


def _build(bs: int, n_cores: int):
    """Build + compile the SPMD Bass program for a batch shard of `bs` rows."""
    import concourse.bass as bass
    import concourse.mybir as mybir
    import concourse.tile as tile
    import concourse.bacc as bacc

    f32 = mybir.dt.float32
    bf16 = mybir.dt.bfloat16
    AF = mybir.ActivationFunctionType

    n_q = bs // QROWS
    assert n_q * QROWS == bs

    nc = bacc.Bacc(
        "TRN2",
        target_bir_lowering=False,
        debug=False,
        num_devices=n_cores,
    )

    x = nc.dram_tensor("x", [bs, F], f32, kind="ExternalInput")
    wz1 = nc.dram_tensor("wz1", [F, H1], f32, kind="ExternalInput")
    b1 = nc.dram_tensor("b1", [1, H1], f32, kind="ExternalInput")
    wz2 = nc.dram_tensor("wz2", [H1, H2], f32, kind="ExternalInput")
    b2 = nc.dram_tensor("b2", [1, H2], f32, kind="ExternalInput")
    wz3 = nc.dram_tensor("wz3", [H2, C], f32, kind="ExternalInput")
    b3 = nc.dram_tensor("b3", [1, C], f32, kind="ExternalInput")
    y = nc.dram_tensor("y", [bs, C], f32, kind="ExternalOutput")

    ident_dram = nc.inline_tensor(np.eye(128, dtype=np.float32), name="ident128")

    # DRAM views
    x_t = x.ap().rearrange("(q c p) f -> q p c f", q=n_q, c=QROWS // 128, p=128)
    wz1_t = wz1.ap().rearrange("(c p) m -> p c m", c=2, p=128)

    with tile.TileContext(nc) as tc:
        with (
            tc.tile_pool(name="const", bufs=1) as const,
            tc.tile_pool(name="xb", bufs=2) as xbp,
            tc.tile_pool(name="xt", bufs=2) as xtp_sb,
            tc.tile_pool(name="h1t", bufs=2) as h1tp,
            tc.tile_pool(name="h2t", bufs=3) as h2tp,
            tc.tile_pool(name="eq", bufs=2) as eqp,
        ):
            # ---- constants / weights (loaded once, cast to bf16 in the DMA) ----
            ident = const.tile([128, 128], bf16)
            nc.gpsimd.dma_start(ident[:], ident_dram.ap())

            wz1_sb = const.tile([128, 2, H1], bf16)
            nc.gpsimd.dma_start(wz1_sb[:], wz1_t)
            wz2_sb = const.tile([H1, H2], bf16)
            nc.gpsimd.dma_start(wz2_sb[:], wz2.ap())
            wz3_sb = const.tile([H2, C], bf16)
            nc.gpsimd.dma_start(wz3_sb[:], wz3.ap())

            b1_sb = const.tile([H1, 1], f32)
            nc.gpsimd.dma_start(b1_sb[:], b1.ap().rearrange("o m -> m o"))
            b2_sb = const.tile([H2, 1], f32)
            nc.gpsimd.dma_start(b2_sb[:], b2.ap().rearrange("o m -> m o"))
            # b3 replicated at partition offsets 0/32/64/96 (junk lanes get 0)
            b3q = const.tile([128, 1], f32)
            nc.vector.memset(b3q[:], 0.0)
            for i in range(4):
                nc.gpsimd.dma_start(
                    b3q[32 * i : 32 * i + C, :], b3.ap().rearrange("o m -> m o")
                )

            ones_k = const.tile([128, 1], f32)
            nc.vector.memset(ones_k[:], 1.0)
            ones_m = const.tile([1, 128], f32)
            nc.vector.memset(ones_m[:], 1.0)

            acc = const.tile([128, n_q], f32)       # exp partial sums per quad
            et_all = const.tile([128, n_q, 512], f32)  # stream-transposed exp

            # ---- main loop over quads of 2048 rows ----
            loop_psum = [
                tc.tile_pool(name="xtpsum", bufs=2, space=bass.MemorySpace.PSUM),
                tc.tile_pool(name="h1psum", bufs=2, space=bass.MemorySpace.PSUM),
                tc.tile_pool(name="h2psum", bufs=2, space=bass.MemorySpace.PSUM),
                tc.tile_pool(name="h3psum", bufs=2, space=bass.MemorySpace.PSUM),
            ]
            xtpp, h1pp, h2pp, h3pp = [p.__enter__() for p in loop_psum]
            for q in range(n_q):
                xb = xbp.tile([128, QROWS // 128, F], bf16, tag="xb")
                nc.gpsimd.dma_start(xb[:], x_t[q])  # f32 -> bf16 cast in DMA

                h3q = h3pp.tile([128, GROUP], f32, tag="h3q")
                nc.vector.memset(h3q[:], -1e30)

                for g in range(GROUPS_PER_Q):
                    # transpose 4 chunks x 2 feature-halves into one bf16 bank
                    xt_ps = xtpp.tile([128, 1024], bf16, tag="xtps")
                    for ci in range(CHUNKS_PER_G):
                        for fh in range(2):
                            nc.tensor.transpose(
                                xt_ps[:, fh * 512 + 128 * ci : fh * 512 + 128 * ci + 128],
                                xb[:, GROUPS_PER_Q * g + ci, 128 * fh : 128 * fh + 128],
                                ident[:],
                            )
                    xt = xtp_sb.tile([128, 1024], bf16, tag="xt")
                    nc.vector.tensor_copy(xt[:], xt_ps[:])

                    h1p = h1pp.tile([H1, GROUP], f32, tag="h1p")
                    nc.tensor.matmul(
                        h1p[:], wz1_sb[:, 0, :], xt[:, 0:512], start=True, stop=False
                    )
                    nc.tensor.matmul(
                        h1p[:], wz1_sb[:, 1, :], xt[:, 512:1024], start=False, stop=True
                    )
                    h1t = h1tp.tile([H1, GROUP], bf16, tag="h1t")
                    nc.scalar.activation(h1t[:], h1p[:], AF.Relu, bias=b1_sb[:, 0:1])

                    h2p = h2pp.tile([H2, GROUP], f32, tag="h2p")
                    nc.tensor.matmul(h2p[:], wz2_sb[:], h1t[:])
                    h2t = h2tp.tile([H2, GROUP], bf16, tag="h2t")
                    nc.scalar.activation(h2t[:], h2p[:], AF.Tanh, bias=b2_sb[:, 0:1])

                    nc.tensor.matmul(
                        h3q[32 * g : 32 * g + C, :],
                        wz3_sb[:],
                        h2t[:],
                        tile_position=(0, 32 * g),
                    )

                eq = eqp.tile([128, GROUP], f32, tag="eq")
                nc.scalar.activation(
                    eq[:], h3q[:], AF.Exp, bias=b3q[:, 0:1],
                    accum_out=acc[:, q : q + 1],
                )
                # 32x32 block transpose: batch back onto partitions
                nc.vector.transpose(et_all[:, q, :], eq[:])

            for p in reversed(loop_psum):
                p.__exit__(None, None, None)

            # ---- global softmax denominator ----
            acc_red = const.tile([128, 1], f32)
            nc.vector.tensor_reduce(
                acc_red[:], acc[:], mybir.AxisListType.X, mybir.AluOpType.add
            )

            with (
                tc.tile_pool(name="spsum", bufs=1, space=bass.MemorySpace.PSUM) as sp,
                tc.tile_pool(name="dram", bufs=1, space=bass.MemorySpace.DRAM) as dram,
            ):
                s_loc_p = sp.tile([1, 1], f32)
                nc.tensor.matmul(s_loc_p[:], acc_red[:], ones_k[:])
                s_loc = const.tile([1, 1], f32)
                nc.vector.tensor_copy(s_loc[:], s_loc_p[:])

                cc_in = dram.tile([1, 1], f32)
                cc_out = dram.tile([1, 1], f32, addr_space="Shared")
                nc.gpsimd.dma_start(cc_in[:], s_loc[:])
                nc.gpsimd.collective_compute(
                    "AllReduce",
                    mybir.AluOpType.add,
                    replica_groups=[list(range(n_cores))],
                    ins=[cc_in.opt()],
                    outs=[cc_out.opt()],
                )
                s_glob = const.tile([1, 1], f32)
                nc.gpsimd.dma_start(s_glob[:], cc_out[:])

                s_bcast = sp.tile([128, 1], f32)
                nc.tensor.matmul(s_bcast[:], ones_m[:], s_glob[:])
                inv_s = const.tile([128, 1], f32)
                nc.vector.reciprocal(inv_s[:], s_bcast[:])

            # ---- scale + write out (natural row layout) ----
            y_ap = y.ap()
            for q in range(n_q):
                nc.vector.tensor_scalar_mul(
                    et_all[:, q, :], et_all[:, q, :], inv_s[:, 0:1]
                )
                for i in range(4):
                    base = QROWS * q + 512 * i
                    src = (
                        et_all[32 * i : 32 * i + 32, q, :]
                        .rearrange("a (j c) -> a j c", c=32)[:, :, 0:C]
                    )
                    dst = y_ap[base : base + 512, :].rearrange(
                        "(j a) c -> a j c", a=32
                    )
                    nc.sync.dma_start(dst, src)

    nc.compile()
    return nc


def _get_nc(bs: int, n_cores: int):
    key = (bs, n_cores)
    if key not in _CACHE:
        _CACHE[key] = _build(bs, n_cores)
    return _CACHE[key]


class _Runner:
    """Cached shard_map runner (mirrors bass2jax.run_bass_via_pjrt, but keeps
    the jitted executable so repeated calls skip retrace/recompile)."""

    def __init__(self, nc):
        import jax
        import jax.numpy as jnp  # noqa: F401
        from jax.sharding import Mesh, PartitionSpec
        from jax.experimental.shard_map import shard_map
        import concourse.mybir as mybir
        from concourse import bass2jax

        bass2jax.install_neuronx_cc_hook()
        self._np = np
        partition_name = (
            nc.partition_id_tensor.name if nc.partition_id_tensor else None
        )
        in_names, out_names, out_avals = [], [], []
        for alloc in nc.m.functions[0].allocations:
            if not isinstance(alloc, mybir.MemoryLocationSet):
                continue
            name = alloc.memorylocations[0].name
            if alloc.kind == "ExternalInput":
                if name != partition_name:
                    in_names.append(name)
            elif alloc.kind == "ExternalOutput":
                out_names.append(name)
                out_avals.append(
                    jax.core.ShapedArray(
                        tuple(alloc.tensor_shape), mybir.dt.np(alloc.dtype)
                    )
                )
        n_params = len(in_names)
        self.in_names = list(in_names)
        self.out_names = out_names
        self.out_avals = out_avals
        all_in = in_names + out_names
        if partition_name is not None:
            all_in = all_in + [partition_name]

        def _body(*args):
            operands = list(args)
            if partition_name is not None:
                operands.append(bass2jax.partition_id_tensor())
            return tuple(
                bass2jax._bass_exec_p.bind(
                    *operands,
                    out_avals=tuple(out_avals),
                    in_names=tuple(all_in),
                    out_names=tuple(out_names),
                    lowering_input_output_aliases=(),
                    sim_require_finite=True,
                    sim_require_nnan=True,
                    nc=nc,
                )
            )

        devices = jax.devices()[:N_CORES]
        mesh = Mesh(np.asarray(devices), ("core",))
        n_outs = len(out_names)
        in_specs = (PartitionSpec("core"),) * (n_params + n_outs)
        out_specs = (PartitionSpec("core"),) * n_outs
        self.sharded = jax.jit(
            shard_map(
                _body, mesh=mesh, in_specs=in_specs, out_specs=out_specs,
                check_rep=False,
            ),
            keep_unused=True,
        )

    def __call__(self, in_maps):
        concat_in = [
            np.concatenate(
                [np.asarray(m[name]) for m in in_maps], axis=0
            )
            for name in self.in_names
        ]
        zeros = [
            np.zeros((N_CORES * a.shape[0], *a.shape[1:]), a.dtype)
            for a in self.out_avals
        ]
        out = self.sharded(*concat_in, *zeros)
        import jax

        out = jax.block_until_ready(out)
        return {
            name: np.asarray(out[i]) for i, name in enumerate(self.out_names)
        }


def _get_runner():
    if "runner" not in _CACHE:
        _CACHE["runner"] = _Runner(_get_nc(BS, N_CORES))
    return _CACHE["runner"]


def _make_in_maps(inputs):
    x = np.ascontiguousarray(inputs["x"], dtype=np.float32)
    common = {
        k: np.ascontiguousarray(inputs[k], dtype=np.float32)
        for k in ("wz1", "b1", "wz2", "b2", "wz3", "b3")
    }
    return [
        {"x": x[i * BS : (i + 1) * BS], **common} for i in range(N_CORES)
    ]


def _run(inputs: dict):
    runner = _get_runner()
    outs = runner(_make_in_maps(inputs))
    return outs["y"], None


def kernel(x, wz1, b1, wz2, b2, wz3, b3):
    out, _ = _run(dict(x=x, wz1=wz1, b1=b1, wz2=wz2, b2=b2, wz3=wz3, b3=b3))
    return out
